# revision 1
# baseline (speedup 1.0000x reference)
"""Trainium2 Bass kernel for nn_CrossAttention (B=8, N=4096, C=512, H=8, d=64).

Math (per batch element b, handled by one NeuronCore):
    kv_j = x_j @ Wkv_j ; k_j, v_j = heads(kv_j)
    ctx_j = scale * k_jh^T v_jh            (per head, [d, d])
          = scale * Wk_jh^T (x_j^T x_j) Wv_jh     <-- Gram trick: G_j = x_j^T x_j
    s_j = softmax(ctx_j, axis over first d)
    out1 = concat_h(q1_h @ s2_h),  out2 = concat_h(q2_h @ s1_h),  q_j = heads(x_j)

The Gram trick halves the dominant matmul FLOPs (G is [512,512] vs the
[4096,1024] kv projection) and removes the need for k/v materialization.

Sharding: batch b -> core b (8 cores, no collectives).
"""

import numpy as np
from contextlib import ExitStack

import concourse.bass as bass
import concourse.tile as tile
from concourse import bacc, mybir, masks
from concourse.bass_utils import run_bass_kernel_spmd

F32 = mybir.dt.float32
F32R = mybir.dt.float32r
BF16 = mybir.dt.bfloat16

B, N, C = 8, 4096, 512
H, D = 8, 64
SCALE = float(D) ** -0.5
TT = 128            # token tile
NTT = N // TT       # 32 token tiles
CK = C // 128       # 4 contraction tiles over C

_CACHE = {}
STOP_AFTER = None  # debug: "G", "ctx", "softmax"


def _emit(tc, io):
    nc = tc.nc
    x_d = [io["x1"], io["x2"]]
    w_d = [io["Wkv1"], io["Wkv2"]]
    o_d = [io["out1"], io["out2"]]

    ctx = ExitStack()
    with ctx:
        pers = ctx.enter_context(tc.tile_pool(name="pers", bufs=1))
        xin = ctx.enter_context(tc.tile_pool(name="xin", bufs=7))
        xtp = ctx.enter_context(tc.tile_pool(name="xtp", bufs=1))
        tsb = ctx.enter_context(tc.tile_pool(name="tsb", bufs=2))
        smp = ctx.enter_context(tc.tile_pool(name="smp", bufs=2))
        outp = ctx.enter_context(tc.tile_pool(name="outp", bufs=7))

        # PSUM pools — every phase sums to exactly 8 banks:
        #   P1:        big4 + scr2            = 6
        #   Tctx0/sm0: big4 + scr2 + ctx0(2)  = 8   -> close ctx0
        #   P2+A2:     big4 + scr2 + out2(2)  = 8   -> close out2
        #   Tctx1/sm1: big4 + scr2 + ctx1(2)  = 8   -> close ctx1
        #   A1:        big4 + scr2 + out1(2)  = 8
        scr_ps = ctx.enter_context(tc.tile_pool(name="scr_ps", bufs=2, space="PSUM"))
        sc_big = ExitStack()
        big_ps = sc_big.enter_context(tc.tile_pool(name="big_ps", bufs=1, space="PSUM"))

        # ---- persistent SBUF ----
        w_sb = pers.tile([128, 2 * CK * 1024], F32R)
        # xT1 (bf16): cols [c*4096 + t*128 : +128] = x1[t*128:+128, 128c:+128].T
        xT1_sb = pers.tile([128, CK * N], BF16)
        g_sb = pers.tile([128, 2 * CK * 512], F32R)
        # S blocks (bf16): tensor j, head-pair k at cols [j*1024 + 128k : +128];
        # quadrants [0:64,0:64]=s_{2k}, [64:128,64:128]=s_{2k+1}, off-diag 0.
        s_sb = pers.tile([128, 2 * 1024], BF16)
        nc.gpsimd.memset(s_sb[:], 0.0)
        ident_f = pers.tile([128, 128], F32)
        masks.make_identity(nc, ident_f[:])
        ident = pers.tile([128, 128], F32R)
        nc.vector.tensor_copy(ident[:], ident_f[:])

        ncopy = [0]

        def eng_copy(dst, src_):
            ncopy[0] += 1
            if ncopy[0] % 3:
                nc.vector.tensor_copy(dst, src_)
            else:
                nc.scalar.activation(dst, src_, mybir.ActivationFunctionType.Copy)

        def load_w_chunk(i):
            j, k = divmod(i, CK)
            nc.sync.dma_start(
                w_sb[:, j * 4096 + k * 1024 : j * 4096 + (k + 1) * 1024],
                w_d[j][128 * k : 128 * (k + 1), :].bitcast(F32R),
            )

        kept_xT = {}

        opair = {}

        def emit_out(j, t, o_ps):
            """Copy a finished out tile into a 2-tile staging buffer; DMA
            every completed pair with one 512KB transfer (halves the
            SWDGE issue count on the gpsimd sequencer)."""
            u, half = divmod(t, 2)
            if half == 0 and t + 1 < NTT:
                o_sb = outp.tile([128, 1024], F32, name="osb", tag="osb")
                opair[(j, u)] = o_sb
            elif (j, u) in opair:
                o_sb = opair[(j, u)]
            else:
                o_sb = outp.tile([128, 1024], F32, name="osb", tag="osb")
                opair[(j, u)] = o_sb
            c0 = 512 * half
            nc.vector.tensor_copy(o_sb[:, c0 : c0 + 320], o_ps[:, 0:320])
            nc.scalar.activation(o_sb[:, c0 + 320 : c0 + 512], o_ps[:, 320:512], mybir.ActivationFunctionType.Copy)
            if half == 1 or t == NTT - 1:
                o_sb = opair.pop((j, u))
                n = 2 if half == 1 else 1
                nc.gpsimd.dma_start(
                    o_d[j][256 * u : 256 * u + 128 * n, :].rearrange("(a p) c -> p a c", p=128),
                    o_sb[:, 0 : 512 * n].rearrange("p (a c) -> p a c", c=512),
                )

        def fused_a2(t, xTt, out_pool):
            o_ps = out_pool.tile([128, 512], F32, name="o2ps", tag="ops")
            for k in range(CK):
                nc.tensor.matmul(
                    o_ps[:, 128 * k : 128 * (k + 1)],
                    xTt[:, 128 * k : 128 * (k + 1)],
                    s_sb[:, 128 * k : 128 * (k + 1)],
                    start=True,
                    stop=True,
                )
            emit_out(1, t, o_ps)

        def stream_phase(j, out_pool, g_ps, t_range, defer_a=()):
            """Stream x_j tiles: G_j accumulation + transposes; for j=1 also
            the fused a2 = q2 @ S1 pass writing out2 (deferred tiles keep
            their xT for a later fused_a2 call)."""
            xpair = {}
            for t in t_range:
                u, half = divmod(t, 2)
                if (j, u) not in xpair:
                    xp = xin.tile([128, 1024], F32R, name="xp", tag="xt")
                    xpair[(j, u)] = xp
                    lo = 2 * u
                    hi = min(2 * u + 2, NTT)
                    n = hi - lo
                    nc.sync.dma_start(
                        xp[:, 0 : 512 * n].rearrange("p (a c) -> p a c", c=512),
                        x_d[j][TT * lo : TT * hi, :].bitcast(F32R).rearrange("(a p) c -> p a c", p=128),
                    )
                xt = xpair[(j, u)][:, 512 * half : 512 * (half + 1)]
                if j == 0 and 4 <= t < 28 and t % 3 == 1:
                    load_w_chunk((t - 4) // 3)
                for m in range(CK):
                    nc.tensor.matmul(
                        g_ps[m][:],
                        xt[:, 128 * m : 128 * (m + 1)],
                        xt[:],
                        start=(t == 0),
                        stop=(t == NTT - 1),
                    )
                if j == 0:
                    xTt = xT1_sb[:, :]
                    off = t * 128
                else:
                    if t in defer_a:
                        xTt = xtp.tile([128, 512], BF16, name="xTt", tag=f"xTk{t}")[:, :]
                    else:
                        xTt = xtp.tile([128, 512], BF16, name="xTt", tag="xTt", bufs=3)[:, :]
                    off = 0
                for c in range(CK):
                    tp = scr_ps.tile([128, 128], F32R, name="tp", tag="scr")
                    nc.tensor.transpose(tp[:], xt[:, 128 * c : 128 * (c + 1)], ident[:])
                    if j == 0:
                        eng_copy(xTt[:, c * 4096 + off : c * 4096 + off + 128], tp[:])
                    else:
                        eng_copy(xTt[:, 128 * c : 128 * (c + 1)], tp[:])
                if j == 1:
                    if t in defer_a:
                        kept_xT[t] = xTt
                    else:
                        fused_a2(t, xTt, out_pool)

        def g_finish(j, g_ps):
            for m in range(CK):
                nc.vector.tensor_copy(
                    g_sb[:, j * 2048 + 512 * m : j * 2048 + 512 * (m + 1)], g_ps[m][:]
                )

        def t_ctx_phase(j, ctx_pool, ctx_ps):
            """ctxT_h = Wv_h^T (G_j Wk_h) for all heads (f32r)."""
            ctx_t = ctx_pool.tile([64, 512], F32, name=f"ctx{j}", tag="ctx")
            ctx_ps[(j, 0)] = ctx_t[0:64, 0:256]
            ctx_ps[(j, 1)] = ctx_t[0:64, 256:512]
            t_ps = {}
            for m in range(CK):
                t_ps[m] = big_ps.tile([128, 512], F32, name=f"tps{m}", tag=f"big{m}")
                for k in range(CK):
                    nc.tensor.matmul(
                        t_ps[m][:],
                        g_sb[:, j * 2048 + 512 * k + 128 * m : j * 2048 + 512 * k + 128 * (m + 1)],
                        w_sb[:, j * 4096 + 1024 * k : j * 4096 + 1024 * k + 512],
                        start=(k == 0),
                        stop=(k == CK - 1),
                    )
            t_sb = tsb.tile([128, 2048], F32R, name="tsb", tag="tsb")
            for m in range(CK):
                if m % 2:
                    nc.vector.tensor_copy(t_sb[:, 512 * m : 512 * (m + 1)], t_ps[m][:])
                else:
                    nc.scalar.activation(
                        t_sb[:, 512 * m : 512 * (m + 1)], t_ps[m][:],
                        mybir.ActivationFunctionType.Copy,
                    )
            for h in range(H):
                cps = ctx_ps[(j, h % 2)]
                q = h // 2
                for k in range(CK):
                    nc.tensor.matmul(
                        cps[:, 64 * q : 64 * (q + 1)],
                        w_sb[:, j * 4096 + 1024 * k + 512 + 64 * h : j * 4096 + 1024 * k + 512 + 64 * (h + 1)],
                        t_sb[:, 512 * k + 64 * h : 512 * k + 64 * (h + 1)],
                        start=(k == 0),
                        stop=(k == CK - 1),
                    )

        def softmax(j, ctx_ps):
            for par in range(2):
                cps = ctx_ps[(j, par)]
                for q in range(4):
                    h = 2 * q + par
                    k = h // 2
                    nmax = smp.tile([64, 1], F32, name="nmax", tag=f"nmax{q}")
                    nc.vector.tensor_reduce(
                        nmax[:], cps[:, 64 * q : 64 * (q + 1)],
                        mybir.AxisListType.X, mybir.AluOpType.max, negate=True,
                    )
                    nbias = smp.tile([64, 1], F32, name="nbias", tag=f"nbias{q}")
                    nc.vector.tensor_scalar_mul(nbias[:], nmax[:], SCALE)
                    expT = smp.tile([64, 64], F32, name="expT", tag=f"expT{q}")
                    accs = smp.tile([64, 1], F32, name="accs", tag=f"accs{q}")
                    nc.scalar.activation(
                        expT[:],
                        cps[:, 64 * q : 64 * (q + 1)],
                        mybir.ActivationFunctionType.Exp,
                        scale=SCALE,
                        bias=nbias[:, 0:1],
                        accum_out=accs[:],
                    )
                    rec = smp.tile([64, 1], F32, name="rec", tag=f"rec{q}")
                    nc.vector.reciprocal(rec[:], accs[:])
                    sT = smp.tile([64, 64], F32R, name="sT", tag=f"sT{q}")
                    nc.vector.tensor_scalar_mul(sT[:], expT[:], rec[:])
                    s_ps = scr_ps.tile([64, 64], F32R, name="sps", tag="scr")
                    nc.tensor.transpose(s_ps[:], sT[:], ident[0:64, 0:64])
                    if par == 0:
                        nc.vector.tensor_copy(
                            s_sb[0:64, j * 1024 + 128 * k : j * 1024 + 128 * k + 64], s_ps[:]
                        )
                    else:
                        stg = smp.tile([64, 64], BF16, name="stg", tag=f"stg{q}")
                        nc.vector.tensor_copy(stg[:], s_ps[:])
                        deng = nc.gpsimd if j == 0 else nc.sync
                        deng.dma_start(
                            s_sb[64:128, j * 1024 + 128 * k + 64 : j * 1024 + 128 * (k + 1)], stg[:]
                        )

        def a1_pass(out_pool):
            for t in range(NTT):
                o_ps = out_pool.tile([128, 512], F32, name="o1ps", tag="ops")
                for k in range(CK):
                    nc.tensor.matmul(
                        o_ps[:, 128 * k : 128 * (k + 1)],
                        xT1_sb[:, k * 4096 + 128 * t : k * 4096 + 128 * (t + 1)],
                        s_sb[:, 1024 + 128 * k : 1024 + 128 * (k + 1)],
                        start=True,
                        stop=True,
                    )
                emit_out(0, t, o_ps)

        ctx_ps = {}
        g1 = [big_ps.tile([128, 512], F32, name=f"g1{m}", tag=f"big{m}") for m in range(CK)]
        stream_phase(0, None, g1, list(range(NTT)))          # G1 + xT1
        g_finish(0, g1)

        sc_ctx0 = ExitStack()
        ctx0_pool = sc_ctx0.enter_context(tc.tile_pool(name="ctx0_ps", bufs=1, space="PSUM"))
        t_ctx_phase(0, ctx0_pool, ctx_ps)                    # ctxT(1)

        # G2 head tiles run on PE while softmax(0)'s DVE/ACT chain computes
        g2 = [big_ps.tile([128, 512], F32, name=f"g2{m}", tag=f"big{m}") for m in range(CK)]
        head = list(range(0, 6))
        tail = list(range(20, NTT))
        stream_phase(1, None, g2, head, defer_a=set(head))
        softmax(0, ctx_ps)                                   # s1
        sc_ctx0.close()

        sc_out2 = ExitStack()
        out2_pool = sc_out2.enter_context(tc.tile_pool(name="out2_ps", bufs=2, space="PSUM"))
        for t in head:
            fused_a2(t, kept_xT.pop(t), out2_pool)
        stream_phase(1, out2_pool, g2, list(range(6, 20)))
        stream_phase(1, None, g2, tail, defer_a=set(tail))
        g_finish(1, g2)
        sc_out2.close()

        sc_ctx1 = ExitStack()
        ctx1_pool = sc_ctx1.enter_context(tc.tile_pool(name="ctx1_ps", bufs=1, space="PSUM"))
        t_ctx_phase(1, ctx1_pool, ctx_ps)                    # ctxT(2)
        # deferred-tail a2 fills softmax(1)'s latency window
        sc_out2b = ExitStack()
        out2b_pool = sc_out2b.enter_context(tc.tile_pool(name="out2b_ps", bufs=1, space="PSUM"))
        for t in tail:
            fused_a2(t, kept_xT.pop(t), out2b_pool)
        sc_out2b.close()
        softmax(1, ctx_ps)                                   # s2
        sc_ctx1.close()
        sc_big.close()

        sc = ExitStack()
        out1_pool = sc.enter_context(tc.tile_pool(name="out1_ps", bufs=5, space="PSUM"))
        a1_pass(out1_pool)                                   # out1 = q1 @ s2
        sc.close()


def _build():
    if "nc" in _CACHE:
        return _CACHE["nc"]
    nc = bacc.Bacc("TRN2", target_bir_lowering=False, debug=False, num_devices=B)
    io = {
        "x1": nc.dram_tensor("x1", [N, C], F32, kind="ExternalInput").ap(),
        "x2": nc.dram_tensor("x2", [N, C], F32, kind="ExternalInput").ap(),
        "Wkv1": nc.dram_tensor("Wkv1", [C, 2 * C], F32, kind="ExternalInput").ap(),
        "Wkv2": nc.dram_tensor("Wkv2", [C, 2 * C], F32, kind="ExternalInput").ap(),
        "out1": nc.dram_tensor("out1", [N, C], F32, kind="ExternalOutput").ap(),
        "out2": nc.dram_tensor("out2", [N, C], F32, kind="ExternalOutput").ap(),
    }

    with tile.TileContext(nc) as tc:
        _emit(tc, io)
    nc.compile()
    _CACHE["nc"] = nc
    return nc


def kernel(x1, x2, Wkv1, Wkv2):
    x1 = np.ascontiguousarray(np.asarray(x1, dtype=np.float32))
    x2 = np.ascontiguousarray(np.asarray(x2, dtype=np.float32))
    Wkv1 = np.ascontiguousarray(np.asarray(Wkv1, dtype=np.float32))
    Wkv2 = np.ascontiguousarray(np.asarray(Wkv2, dtype=np.float32))

    nc = _build()
    in_maps = [
        {"x1": x1[b], "x2": x2[b], "Wkv1": Wkv1, "Wkv2": Wkv2} for b in range(B)
    ]
    res = run_bass_kernel_spmd(nc, in_maps, list(range(B))).results
    out1 = np.stack([res[b]["out1"] for b in range(B)])
    out2 = np.stack([res[b]["out2"] for b in range(B)])
    return out1, out2


if __name__ == "__main__":
    rng = np.random.default_rng(0)
    o1, o2 = kernel(
        rng.standard_normal((B, N, C), dtype=np.float32),
        rng.standard_normal((B, N, C), dtype=np.float32),
        rng.standard_normal((C, 2 * C), dtype=np.float32) * C**-0.5,
        rng.standard_normal((C, 2 * C), dtype=np.float32) * C**-0.5,
    )
    print(o1.shape, o2.shape)

